# revision 1
# baseline (speedup 1.0000x reference)
"""Trainium2 Bass kernel for the GCM aspect-sentiment model.

Sharding: pure data parallelism — batch (32) split across 8 NeuronCores
(4 items/core); embedding table + all weights replicated.

Per-core plan (all matmuls bf16 with fp32 PSUM accumulation):
  - embedding rows gathered from DRAM via indirect DMA, cast bf16,
    PE-transposed to channel-major [D, B*L] padded layout
  - attention via 2nd-order expansion of tanh(cw+aw) in the small aspect
    term: score = U.ty - (V tx (1-tx^2)).ty^2 with U = V(1-tx^2); the
    l-constant term V.tx cancels in softmax.  This removes the
    [B,L1,L2,E] tanh entirely.
  - conv1/conv2 as 3-tap shifted matmuls; asp_w and asp_b folded into the
    aspect half of conv2 on the host.
  - highway + maxpool + classifier on-chip.

Layout rules learned on HW:
  - every vector/scalar-engine SBUF write lands on a 256B-aligned column
    (2B-aligned writes take a ~70us slow path); conv halo columns
    (l = -1, l = 512) read zero padding instead of shifting data by one.
  - per-b activation blocks are strided LPB=640 cols with data at +128.
  - D is split 100/100/100 so all three transposed chunks share one PSUM
    tile and spill to SBUF with a single strided copy per token tile.
  - per-b matmuls accumulate into 512-col slices of 2-bank PSUM tiles
    (b-pairs) so post-ops (tanh/relu/elementwise) batch over 2 b at once;
    sections are ordered h-outer so downstream stages start early.
"""

import numpy as np
import ml_dtypes

import concourse.bacc as bacc
import concourse.mybir as mybir
import concourse.tile as tile
from concourse.bass import IndirectOffsetOnAxis
from concourse.masks import make_identity
from concourse.bass_utils import run_bass_kernel_spmd

B, L1, L2 = 32, 512, 16
D, C, NCLS = 300, 256, 3
K = 3
VOCAB = 50000
E = D + C
NCORES = 8
BL = B // NCORES          # batch per core
NL = BL * L1              # 2048 context tokens per core
NM = BL * L2              # 64 aspect tokens per core
LPB = 640                 # per-b block stride in ctxT/attT (data at +128)
CTW = BL * LPB + 128      # per-D-chunk block width in ctxT
ASB = 128                 # per-b block stride in aspT (data at (b+1)*128)
ASW = (BL + 2) * 128      # per-D-chunk block width in aspT

bf16 = mybir.dt.bfloat16
f32 = mybir.dt.float32
i32 = mybir.dt.int32
AF = mybir.ActivationFunctionType
ALU = mybir.AluOpType
AX = mybir.AxisListType
np_bf16 = ml_dtypes.bfloat16

DSZ = 100
ND = 3                     # number of D chunks
HB = 2                     # batch items per big PSUM tile (2 banks)
LH = HB * L1               # 1024
D_TILES = [(0, 100), (100, 100), (200, 100)]
C_TILES = [(0, 128), (128, 128)]
E_TILES = [(0, 128), (128, 128), (256, 128), (384, 128), (512, 44)]

_NC_CACHE = {}


def build_nc(stage=None, repeat=1):
    nc = bacc.Bacc("TRN2", target_bir_lowering=False, debug=False)

    # ---- DRAM I/O ----
    d_ctx_ids = nc.dram_tensor("ctx_ids", [NL, 1], i32, kind="ExternalInput")
    d_asp_ids = nc.dram_tensor("asp_ids", [NM, 1], i32, kind="ExternalInput")
    d_emb = nc.dram_tensor("wordemb", [VOCAB, D], bf16, kind="ExternalInput")
    d_w1t = nc.dram_tensor("w1t", [D, E], bf16, kind="ExternalInput")
    d_w2t = nc.dram_tensor("w2t", [C, E], bf16, kind="ExternalInput")
    d_v2 = nc.dram_tensor("v2", [E, 2], f32, kind="ExternalInput")
    d_w3 = nc.dram_tensor("w3t", [D, K * C], bf16, kind="ExternalInput")
    d_w1c = nc.dram_tensor("w1ct", [D, K * C], bf16, kind="ExternalInput")
    d_w2ctx = nc.dram_tensor("w2ctxt", [D, K * C], bf16, kind="ExternalInput")
    d_w2att = nc.dram_tensor("w2attt", [C, K * C], bf16, kind="ExternalInput")
    d_hwt = nc.dram_tensor("hwt", [C, C], bf16, kind="ExternalInput")
    d_hwgt = nc.dram_tensor("hwgt", [C, C], bf16, kind="ExternalInput")
    d_outwt = nc.dram_tensor("outwt", [C, NCLS], bf16, kind="ExternalInput")
    d_bias = nc.dram_tensor("biases", [C, 5], f32, kind="ExternalInput")
    d_outb = nc.dram_tensor("outb", [BL, NCLS], f32, kind="ExternalInput")
    d_out = nc.dram_tensor("out", [BL, NCLS], f32, kind="ExternalOutput")

    with tile.TileContext(nc) as tc:
        for _rep in range(repeat):
            _body(nc, tc, d_ctx_ids, d_asp_ids, d_emb, d_w1t, d_w2t, d_v2,
                  d_w3, d_w1c, d_w2ctx, d_w2att, d_hwt, d_hwgt, d_outwt,
                  d_bias, d_outb, d_out, stage=stage)
    nc.compile()
    return nc


def _body(nc, tc, d_ctx_ids, d_asp_ids, d_emb, d_w1t, d_w2t, d_v2, d_w3,
          d_w1c, d_w2ctx, d_w2att, d_hwt, d_hwgt, d_outwt, d_bias, d_outb,
          d_out, stage=None):
    import contextlib
    stack = contextlib.ExitStack()
    cst = stack.enter_context(tc.tile_pool(name="cst", bufs=1))
    per = stack.enter_context(tc.tile_pool(name="per", bufs=1))
    wk = stack.enter_context(tc.tile_pool(name="wk", bufs=3))
    ps2 = stack.enter_context(tc.tile_pool(name="ps2", bufs=2, space="PSUM"))
    pbig = stack.enter_context(tc.tile_pool(name="pbig", bufs=3, space="PSUM"))

    def finish(src):
        osb = wk.tile([BL, NCLS], f32, tag="osb", name="osb")
        nc.vector.tensor_copy(osb[:], src)
        nc.sync.dma_start(d_out.ap(), osb[:])
        stack.close()

    # ---- constants into SBUF ----
    ident = cst.tile([128, 128], bf16, tag="ident", name="ident")
    make_identity(nc, ident[:])

    # token-id DMAs first: the embedding gathers head the critical path
    # and must not queue behind ~30 weight-load DMAs.
    idxa = wk.tile([NM, 1], i32, tag="idxa", name="idxa")
    nc.sync.dma_start(idxa[:], d_asp_ids.ap())
    idx16 = wk.tile([128, NL // 128], i32, tag="idx16", name="idx16")
    nc.sync.dma_start(
        idx16[:], d_ctx_ids.ap().rearrange("(t p) o -> p (t o)", p=128))

    def load_conv_w(dram, tiles, name):
        out = []
        for it, (o0, osz) in enumerate(tiles):
            t = cst.tile([osz, K * C], bf16, tag=f"{name}{it}", name=f"{name}{it}")
            nc.sync.dma_start(t[:], dram.ap()[o0:o0 + osz, :])
            out.append(t)
        return out

    w3_sb = load_conv_w(d_w3, D_TILES, "w3")
    w2t_sb = []
    for ct, (c0, csz) in enumerate(C_TILES):
        t = cst.tile([csz, E], bf16, tag=f"w2t{ct}", name=f"w2t{ct}")
        nc.sync.dma_start(t[:], d_w2t.ap()[c0:c0 + csz, :])
        w2t_sb.append(t)
    w1t_sb = []
    for dt, (d0, dsz) in enumerate(D_TILES):
        t = cst.tile([dsz, E], bf16, tag=f"w1t{dt}", name=f"w1t{dt}")
        nc.sync.dma_start(t[:], d_w1t.ap()[d0:d0 + dsz, :])
        w1t_sb.append(t)
    v2_sb = []
    for et, (e0, esz) in enumerate(E_TILES):
        t = cst.tile([esz, 2], f32, tag=f"v2{et}", name=f"v2{et}")
        nc.sync.dma_start(t[:], d_v2.ap()[e0:e0 + esz, :])
        v2_sb.append(t)
    w1c_sb = load_conv_w(d_w1c, D_TILES, "w1c")
    w2ctx_sb = load_conv_w(d_w2ctx, D_TILES, "w2ctx")
    w2att_sb = load_conv_w(d_w2att, C_TILES, "w2att")

    hwt_sb, hwgt_sb, outwt_sb, bias_sb = [], [], [], []
    for ct, (c0, csz) in enumerate(C_TILES):
        t = cst.tile([csz, C], bf16, tag=f"hwt{ct}", name=f"hwt{ct}")
        nc.sync.dma_start(t[:], d_hwt.ap()[c0:c0 + csz, :])
        hwt_sb.append(t)
        t = cst.tile([csz, C], bf16, tag=f"hwgt{ct}", name=f"hwgt{ct}")
        nc.sync.dma_start(t[:], d_hwgt.ap()[c0:c0 + csz, :])
        hwgt_sb.append(t)
        t = cst.tile([csz, NCLS], bf16, tag=f"outwt{ct}", name=f"outwt{ct}")
        nc.sync.dma_start(t[:], d_outwt.ap()[c0:c0 + csz, :])
        outwt_sb.append(t)
        t = cst.tile([csz, 5], f32, tag=f"bias{ct}", name=f"bias{ct}")
        nc.sync.dma_start(t[:], d_bias.ap()[c0:c0 + csz, :])
        bias_sb.append(t)
    outb_sb = cst.tile([BL, NCLS], f32, tag="outb", name="outb")
    nc.sync.dma_start(outb_sb[:], d_outb.ap())

    # ---- persistent activations ----
    # ctxT/aspT: one tile, D chunks as column blocks (all chunks 100 rows)
    ctxT = per.tile([DSZ, ND * CTW], bf16, tag="ctxT", name="ctxT")
    aspT = per.tile([DSZ, ND * ASW], bf16, tag="aspT", name="aspT")
    attT = [per.tile([csz, CTW], bf16, tag=f"attT{ct}", name=f"attT{ct}")
            for ct, (c0, csz) in enumerate(C_TILES)]

    def ctx_mv(dt, base):
        """[DSZ, 512] view of ctxT chunk dt starting at block col `base`."""
        return ctxT[:, dt * CTW + base:dt * CTW + base + L1]

    # memset only the padding columns (pad blocks of 128 cols, stride 640)
    for dt in range(ND):
        pad = ctxT[:, dt * CTW:dt * CTW + BL * LPB].rearrange(
            "p (z w) -> p z w", w=LPB)
        nc.gpsimd.memset(pad[:, :, 0:128], 0.0)
        nc.gpsimd.memset(ctxT[:, dt * CTW + BL * LPB:(dt + 1) * CTW], 0.0)
    nc.gpsimd.memset(aspT[:], 0.0)
    for ct, (c0, csz) in enumerate(C_TILES):
        pad = attT[ct][:, 0:BL * LPB].rearrange("p (z w) -> p z w", w=LPB)
        nc.gpsimd.memset(pad[:, :, 0:128], 0.0)
        nc.gpsimd.memset(attT[ct][:, BL * LPB:CTW], 0.0)

    UT = [per.tile([esz, NL], bf16, tag=f"UT{et}", name=f"UT{et}")
          for et, (e0, esz) in enumerate(E_TILES)]
    U2T = [per.tile([esz, NL], bf16, tag=f"U2T{et}", name=f"U2T{et}")
           for et, (e0, esz) in enumerate(E_TILES)]
    tyT = [per.tile([esz, NM], bf16, tag=f"tyT{et}", name=f"tyT{et}")
           for et, (e0, esz) in enumerate(E_TILES)]
    nty2T = [per.tile([esz, NM], bf16, tag=f"nty2T{et}", name=f"nty2T{et}")
             for et, (e0, esz) in enumerate(E_TILES)]
    aT = [per.tile([csz, NM], bf16, tag=f"aT{ct}", name=f"aT{ct}")
          for ct, (c0, csz) in enumerate(C_TILES)]
    a_b = [per.tile([L2, C], bf16, tag=f"a_b{b}", name=f"a_b{b}") for b in range(BL)]
    alphaT = per.tile([L2, NL], bf16, tag="alphaT", name="alphaT")
    mT = [per.tile([csz, NL], bf16, tag=f"mT{ct}", name=f"mT{ct}")
          for ct, (c0, csz) in enumerate(C_TILES)]
    pooled = [[per.tile([csz, HB], bf16, tag=f"pl{ct}_{h}", name=f"pl{ct}_{h}")
               for h in range(BL // HB)] for ct, (c0, csz) in enumerate(C_TILES)]

    if stage == 0:
        return finish(ident[0:BL, 0:NCLS])

    # ---- aspect branch ----
    gab = wk.tile([NM, D], bf16, tag="gathab", name="gathab")
    nc.gpsimd.indirect_dma_start(
        out=gab[:], out_offset=None, in_=d_emb.ap(),
        in_offset=IndirectOffsetOnAxis(ap=idxa[:, 0:1], axis=0))
    trb = ps2.tile([DSZ, ND * 128], bf16, tag="sm", name="trb")
    for dt, (d0, dsz) in enumerate(D_TILES):
        nc.tensor.transpose(out=trb[:, dt * 128:dt * 128 + NM],
                            in_=gab[:, d0:d0 + dsz],
                            identity=ident[:NM, :NM])
    for b in range(BL):
        dst = aspT[:].rearrange("p (dt w) -> p dt w", w=ASW)[
            :, :, (b + 1) * ASB:(b + 1) * ASB + L2]
        src = trb[:].rearrange("p (dt w) -> p dt w", w=128)[:, :, b * L2:(b + 1) * L2]
        nc.vector.tensor_copy(dst, src)

    # conv3 + relu -> aT  (out view [c, b, m]); moving operand reads the
    # aspT blocks at (b+1)*128 + k - 1 via a rearrange based at col 127.
    for ct, (c0, csz) in enumerate(C_TILES):
        pa = ps2.tile([128, NM], f32, tag="sm", name="pa")
        pa_v = pa[:csz, :].rearrange("p (b m) -> p b m", m=L2)
        first = True
        for k in range(K):
            for dt, (d0, dsz) in enumerate(D_TILES):
                rhs = aspT[:, dt * ASW + ASB - 1:dt * ASW + ASB - 1 + BL * ASB] \
                    .rearrange("p (b w) -> p b w", w=ASB)[:, :, k:k + L2]
                nc.tensor.matmul(pa_v, w3_sb[dt][:, k * C + c0:k * C + c0 + csz],
                                 rhs, start=first,
                                 stop=(k == K - 1 and dt == len(D_TILES) - 1))
                first = False
        nc.scalar.activation(aT[ct][:], pa[:csz, :], AF.Relu,
                             bias=bias_sb[ct][:, 0:1])
    # a_b: per-batch [m, c]; both ct transposes share a tile, one copy
    for b in range(BL):
        tr = ps2.tile([128, C], bf16, tag="sm", name="tr")
        for ct, (c0, csz) in enumerate(C_TILES):
            nc.tensor.transpose(out=tr[:L2, c0:c0 + csz],
                                in_=aT[ct][:, b * L2:(b + 1) * L2],
                                identity=ident[:csz, :csz])
        nc.vector.tensor_copy(a_b[b][:], tr[:L2, :])
    # aw -> ty, -ty^2
    for et, (e0, esz) in enumerate(E_TILES):
        paw = ps2.tile([128, NM], f32, tag="sm", name="paw")
        for ct, (c0, csz) in enumerate(C_TILES):
            nc.tensor.matmul(paw[:esz, :], w2t_sb[ct][:, e0:e0 + esz], aT[ct][:],
                             start=(ct == 0), stop=(ct == len(C_TILES) - 1))
        nc.scalar.activation(tyT[et][:], paw[:esz, :], AF.Tanh)
        ty2 = wk.tile([128, NM], bf16, tag="ty2", name="ty2")
        nc.scalar.activation(ty2[:esz, :], tyT[et][:], AF.Square)
        nc.vector.tensor_scalar_mul(nty2T[et][:], ty2[:esz, :], -1.0)

    # ---- context gather + transpose ----
    for t in range(NL // 128):
        b, lc = t // 4, t % 4
        gb = wk.tile([128, D], bf16, tag="gathb", name="gathb")
        nc.gpsimd.indirect_dma_start(
            out=gb[:], out_offset=None, in_=d_emb.ap(),
            in_offset=IndirectOffsetOnAxis(ap=idx16[:, t:t + 1], axis=0))
        trb = ps2.tile([DSZ, ND * 128], bf16, tag="sm", name="trb")
        for dt, (d0, dsz) in enumerate(D_TILES):
            nc.tensor.transpose(out=trb[:, dt * 128:(dt + 1) * 128],
                                in_=gb[:, d0:d0 + dsz], identity=ident[:])
        col = b * LPB + 128 + lc * 128
        dst = ctxT[:].rearrange("p (dt w) -> p dt w", w=CTW)[:, :, col:col + 128]
        src = trb[:].rearrange("p (dt w) -> p dt w", w=128)
        nc.vector.tensor_copy(dst, src)

    if stage == 1:
        return finish(ctxT[0:BL, 0:NCLS])

    if stage == 2:
        return finish(tyT[0][0:BL, 0:NCLS])

    # ---- cw -> tx -> U, U2  (per et; b-pairs batched via 2-bank PSUM) ----
    for h in range(BL // HB):
        for et, (e0, esz) in enumerate(E_TILES):
            pcw = pbig.tile([128, LH], f32, tag="big", name="pcw")
            for j in range(HB):
                b = h * HB + j
                for dt, (d0, dsz) in enumerate(D_TILES):
                    nc.tensor.matmul(pcw[:esz, j * L1:(j + 1) * L1],
                                     w1t_sb[dt][:, e0:e0 + esz],
                                     ctx_mv(dt, b * LPB + 128),
                                     start=(dt == 0), stop=(dt == len(D_TILES) - 1))
            hs = slice(h * LH, (h + 1) * LH)
            tx = wk.tile([128, LH], bf16, tag="tx", name="tx")
            nc.scalar.activation(tx[:esz, :], pcw[:esz, :], AF.Tanh)
            sq = wk.tile([128, LH], bf16, tag="sq", name="sq")
            nc.scalar.activation(sq[:esz, :], tx[:esz, :], AF.Square)
            nc.vector.tensor_scalar(UT[et][:, hs], sq[:esz, :],
                                    v2_sb[et][:, 1:2], v2_sb[et][:, 0:1],
                                    op0=ALU.mult, op1=ALU.add)
            nc.vector.tensor_tensor(U2T[et][:, hs], tx[:esz, :], UT[et][:, hs],
                                    op=ALU.mult)

    if stage == 3:
        return finish(UT[0][0:BL, 0:NCLS])

    # ---- score -> softmax -> alphaT (4 lc regions share one PSUM tile) ----
    NLC = L1 // 128
    for b in range(BL):
        psc = ps2.tile([128, NLC * L2], f32, tag="sm", name="sc")
        n_et = len(E_TILES)
        for lc in range(NLC):
            col = b * L1 + lc * 128
            reg = psc[:, lc * L2:(lc + 1) * L2]
            for et, (e0, esz) in enumerate(E_TILES):
                nc.tensor.matmul(reg, UT[et][:esz, col:col + 128],
                                 tyT[et][:, b * L2:(b + 1) * L2],
                                 start=(et == 0), stop=False)
            for et, (e0, esz) in enumerate(E_TILES):
                nc.tensor.matmul(reg, U2T[et][:esz, col:col + 128],
                                 nty2T[et][:, b * L2:(b + 1) * L2],
                                 start=False, stop=(et == n_et - 1))
        al_u = wk.tile([128, NLC * L2], bf16, tag="alu", name="alu")
        nc.scalar.activation(al_u[:], psc[:], AF.Exp)
        rs4 = wk.tile([128, NLC], f32, tag="rs4", name="rs4")
        nc.vector.reduce_sum(
            out=rs4[:], in_=al_u[:].rearrange("p (z m) -> p z m", m=L2),
            axis=AX.X)
        rc4 = wk.tile([128, NLC], f32, tag="rc4", name="rc4")
        nc.vector.reciprocal(rc4[:], rs4[:])
        als = []
        for lc in range(NLC):
            al = wk.tile([128, L2], bf16, tag=f"al{lc}", name=f"al{lc}")
            nc.vector.tensor_scalar_mul(al[:], al_u[:, lc * L2:(lc + 1) * L2],
                                        rc4[:, lc:lc + 1])
            als.append(al)
        trb4 = ps2.tile([128, L1], bf16, tag="sm", name="trb4")
        for lc in range(NLC):
            nc.tensor.transpose(out=trb4[:L2, lc * 128:(lc + 1) * 128],
                                in_=als[lc][:], identity=ident[:])
        nc.vector.tensor_copy(alphaT[:, b * L1:(b + 1) * L1], trb4[:L2, :])

    if stage == 4:
        return finish(alphaT[0:BL, 0:NCLS])

    # ---- att (normalized); copies strided into per-b blocks ----
    for h in range(BL // HB):
        for ct, (c0, csz) in enumerate(C_TILES):
            pat = pbig.tile([128, LH], f32, tag="big", name="pat")
            for j in range(HB):
                b = h * HB + j
                nc.tensor.matmul(pat[:csz, j * L1:(j + 1) * L1],
                                 a_b[b][:, c0:c0 + csz],
                                 alphaT[:, b * L1:(b + 1) * L1],
                                 start=True, stop=True)
            dst = attT[ct][:, h * HB * LPB + 128:].rearrange(
                "p (z w) -> p z w", w=LPB)[:, 0:HB, 0:L1]
            nc.vector.tensor_copy(
                dst, pat[:csz, :].rearrange("p (z w) -> p z w", w=L1))

    # ---- conv1 (tanh) and conv2 (relu, asp folded) -> m ----
    for h in range(BL // HB):
        for ct, (c0, csz) in enumerate(C_TILES):
            hs = slice(h * LH, (h + 1) * LH)
            ps1 = pbig.tile([128, LH], f32, tag="big", name="ps1")
            for j in range(HB):
                b = h * HB + j
                first = True
                for k in range(K):
                    for dt, (d0, dsz) in enumerate(D_TILES):
                        nc.tensor.matmul(
                            ps1[:csz, j * L1:(j + 1) * L1],
                            w1c_sb[dt][:, k * C + c0:k * C + c0 + csz],
                            ctx_mv(dt, b * LPB + 127 + k),
                            start=first,
                            stop=(k == K - 1 and dt == len(D_TILES) - 1))
                        first = False
            s1 = wk.tile([128, LH], bf16, tag="s1", name="s1")
            nc.scalar.activation(s1[:csz, :], ps1[:csz, :], AF.Tanh,
                                 bias=bias_sb[ct][:, 1:2])
            pg = pbig.tile([128, LH], f32, tag="big", name="pg")
            for j in range(HB):
                b = h * HB + j
                first = True
                for k in range(K):
                    for dt, (d0, dsz) in enumerate(D_TILES):
                        nc.tensor.matmul(
                            pg[:csz, j * L1:(j + 1) * L1],
                            w2ctx_sb[dt][:, k * C + c0:k * C + c0 + csz],
                            ctx_mv(dt, b * LPB + 127 + k),
                            start=first, stop=False)
                        first = False
                for k in range(K):
                    for jt, (j0, jsz) in enumerate(C_TILES):
                        nc.tensor.matmul(
                            pg[:csz, j * L1:(j + 1) * L1],
                            w2att_sb[jt][:, k * C + c0:k * C + c0 + csz],
                            attT[jt][:, b * LPB + 127 + k:
                                      b * LPB + 127 + k + L1],
                            start=False,
                            stop=(k == K - 1 and jt == len(C_TILES) - 1))
            gg = wk.tile([128, LH], bf16, tag="gg", name="gg")
            nc.scalar.activation(gg[:csz, :], pg[:csz, :], AF.Relu,
                                 bias=bias_sb[ct][:, 2:3])
            nc.vector.tensor_tensor(mT[ct][:, hs], s1[:csz, :], gg[:csz, :],
                                    op=ALU.mult)

    if stage == 5:
        return finish(mT[0][0:BL, 0:NCLS])

    # ---- highway + maxpool + per-b classifier ----
    prow = wk.tile([BL, NCLS], f32, tag="prow", name="prow")
    for h in range(BL // HB):
        for ct, (c0, csz) in enumerate(C_TILES):
            hs = slice(h * LH, (h + 1) * LH)
            ph = pbig.tile([128, LH], f32, tag="big", name="ph")
            for j in range(HB):
                b = h * HB + j
                for jt, (j0, jsz) in enumerate(C_TILES):
                    nc.tensor.matmul(ph[:csz, j * L1:(j + 1) * L1],
                                     hwt_sb[jt][:, c0:c0 + csz],
                                     mT[jt][:, b * L1:(b + 1) * L1],
                                     start=(jt == 0), stop=(jt == len(C_TILES) - 1))
            hh = wk.tile([128, LH], bf16, tag="hh", name="hh")
            nc.scalar.activation(hh[:csz, :], ph[:csz, :], AF.Relu,
                                 bias=bias_sb[ct][:, 3:4])
            phg = pbig.tile([128, LH], f32, tag="big", name="phg")
            for j in range(HB):
                b = h * HB + j
                for jt, (j0, jsz) in enumerate(C_TILES):
                    nc.tensor.matmul(phg[:csz, j * L1:(j + 1) * L1],
                                     hwgt_sb[jt][:, c0:c0 + csz],
                                     mT[jt][:, b * L1:(b + 1) * L1],
                                     start=(jt == 0), stop=(jt == len(C_TILES) - 1))
            gt = wk.tile([128, LH], bf16, tag="gt", name="gt")
            nc.scalar.activation(gt[:csz, :], phg[:csz, :], AF.Sigmoid,
                                 bias=bias_sb[ct][:, 4:5])
            dd = wk.tile([128, LH], bf16, tag="dd", name="dd")
            nc.vector.tensor_tensor(dd[:csz, :], hh[:csz, :], mT[ct][:, hs],
                                    op=ALU.subtract)
            ee = wk.tile([128, LH], bf16, tag="ee", name="ee")
            nc.vector.tensor_tensor(ee[:csz, :], gt[:csz, :], dd[:csz, :],
                                    op=ALU.mult)
            m2 = wk.tile([128, LH], bf16, tag="m2", name="m2")
            nc.vector.tensor_tensor(m2[:csz, :], ee[:csz, :], mT[ct][:, hs],
                                    op=ALU.add)
            nc.vector.reduce_max(
                out=pooled[ct][h][:],
                in_=m2[:csz, :].rearrange("p (z w) -> p z w", w=L1),
                axis=AX.X)
        for j in range(HB):
            b = h * HB + j
            po = ps2.tile([128, L2], f32, tag="sm", name="po")
            for ct, (c0, csz) in enumerate(C_TILES):
                nc.tensor.matmul(po[0:1, :NCLS], pooled[ct][h][:, j:j + 1],
                                 outwt_sb[ct][:],
                                 start=(ct == 0), stop=(ct == len(C_TILES) - 1))
            pr1 = wk.tile([1, NCLS], f32, tag="pr1", name="pr1")
            nc.vector.tensor_copy(pr1[:], po[0:1, :NCLS])
            nc.sync.dma_start(prow[b:b + 1, :], pr1[:])

    # ---- output assembly (classifier rows DMA'd per-b inside highway) ----
    osb = wk.tile([BL, NCLS], f32, tag="osb", name="osb")
    nc.vector.tensor_tensor(osb[:], prow[:], outb_sb[:], op=ALU.add)
    nc.sync.dma_start(d_out.ap(), osb[:])
    stack.close()


def prep_inputs(context_ids, aspect_ids, wordemb, conv3_w, conv3_b, conv1_w,
                conv1_b, conv2_w, conv2_b, attn_W, attn_V, asp_w, asp_b, hw_w,
                hw_b, hwg_w, hwg_b, out_w, out_b):
    """Host-side prep: weight layout transforms + bf16 casts (weights only)."""
    f = np.float32
    attn_W = np.asarray(attn_W, f)
    w2 = np.asarray(conv2_w, f)
    asp_w = np.asarray(asp_w, f)

    shared = {
        "wordemb": np.asarray(wordemb, f).astype(np_bf16),
        "w1t": np.ascontiguousarray(attn_W[:, :D].T).astype(np_bf16),
        "w2t": np.ascontiguousarray(attn_W[:, D:].T).astype(np_bf16),
        "v2": np.stack([np.asarray(attn_V, f)[0], -np.asarray(attn_V, f)[0]],
                       axis=1).astype(f),
        "w3t": np.asarray(conv3_w, f).transpose(1, 2, 0).reshape(D, K * C)
              .astype(np_bf16),
        "w1ct": np.asarray(conv1_w, f).transpose(1, 2, 0).reshape(D, K * C)
               .astype(np_bf16),
        "w2ctxt": w2[:, :D, :].transpose(1, 2, 0).reshape(D, K * C)
                 .astype(np_bf16),
        "w2attt": np.einsum("aok,oc->ack", w2[:, D:, :], asp_w)
                 .transpose(1, 2, 0).reshape(C, K * C).astype(np_bf16),
        "hwt": np.ascontiguousarray(np.asarray(hw_w, f).T).astype(np_bf16),
        "hwgt": np.ascontiguousarray(np.asarray(hwg_w, f).T).astype(np_bf16),
        "outwt": np.ascontiguousarray(np.asarray(out_w, f).T).astype(np_bf16),
        "biases": np.stack([
            np.asarray(conv3_b, f),
            np.asarray(conv1_b, f),
            np.asarray(conv2_b, f) + np.einsum("aok,o->a", w2[:, D:, :],
                                               np.asarray(asp_b, f)),
            np.asarray(hw_b, f),
            np.asarray(hwg_b, f)], axis=1).astype(f),
        "outb": np.tile(np.asarray(out_b, f).reshape(1, NCLS), (BL, 1)),
    }
    in_maps = []
    for c in range(NCORES):
        m = dict(shared)
        m["ctx_ids"] = np.ascontiguousarray(
            np.asarray(context_ids, np.int32)[c * BL:(c + 1) * BL]
        ).reshape(NL, 1)
        m["asp_ids"] = np.ascontiguousarray(
            np.asarray(aspect_ids, np.int32)[c * BL:(c + 1) * BL]
        ).reshape(NM, 1)
        in_maps.append(m)
    return in_maps


def kernel(**inputs):
    if "nc" not in _NC_CACHE:
        _NC_CACHE["nc"] = build_nc()
    nc = _NC_CACHE["nc"]
    in_maps = prep_inputs(**inputs)
    res = run_bass_kernel_spmd(nc, in_maps, core_ids=list(range(NCORES)))
    return np.concatenate([res.results[c]["out"] for c in range(NCORES)], axis=0)


if __name__ == "__main__":
    rng = np.random.default_rng(0)
    print("building...")
    nc = build_nc()
    print("built ok")



# revision 68
# speedup vs baseline: 1.1445x; 1.1445x over previous
"""Trainium2 Bass kernel for the GCM aspect-sentiment model.

Sharding: pure data parallelism — batch (32) split across 8 NeuronCores
(4 items/core); embedding table + all weights replicated.

Per-core plan (all matmuls bf16 with fp32 PSUM accumulation):
  - embedding rows gathered from DRAM via indirect DMA (one 128-row
    gather per token tile; [<=128, 1] offset columns only), cast bf16,
    PE-transposed to channel-major dense [D, B*L] layout.
  - attention via 1st-order expansion of tanh(cw+aw) in the small aspect
    term: score = (V(1-tx^2)).ty; the l-constant term V.tx cancels in
    softmax and the ty^2 term is below bf16 noise (verified on host).
  - convs as shifted-tap matmuls: tap k accumulates into a shifted
    column range of the same PSUM bank, so zero-padding needs no halo
    columns or memsets, and activation layouts are fully dense.
  - weights land via packed [128, W] tensors; chunks are issued on the
    SP queue (early consumers) or interleaved into the Pool gather
    stream (the Pool DGE drains each DMA through its transfer on a FIFO
    bus, so ordering is placement).
  - conv1 and conv2(h0) interleaved into the attention phase (PE
    otherwise idles there while Act/DVE chew tanh/softmax chains, and
    idle PE drops to a lower p-state clock).
  - highway + per-b maxpool + classifier batched into one PSUM tile,
    single bias add, single output DMA.

Constraints learned on HW:
  - every vector/scalar-engine SBUF write lands on a 256B-aligned column
    (2B-aligned writes take a ~70us slow path);
  - matmul out/moving free size <= 512 (one PSUM bank), out base
    partition in {0, 32, 64};
  - tensor_tensor_reduce with op1=max hangs the device; gpsimd cannot
    touch PSUM; fp8 on conv1/conv2 breaks the 2e-2 gate (host-verified).
"""

import contextlib

import numpy as np
import ml_dtypes

import concourse.bacc as bacc
import concourse.mybir as mybir
import concourse.tile as tile
from concourse.bass import IndirectOffsetOnAxis
from concourse.bass_utils import run_bass_kernel_spmd

B, L1, L2 = 32, 512, 16
D, C, NCLS = 300, 256, 3
K = 3
VOCAB = 50000
E = D + C
NCORES = 8
BL = B // NCORES          # batch per core
NL = BL * L1              # 2048 context tokens per core
NM = BL * L2              # 64 aspect tokens per core

bf16 = mybir.dt.bfloat16
f32 = mybir.dt.float32
i32 = mybir.dt.int32
AF = mybir.ActivationFunctionType
ALU = mybir.AluOpType
AX = mybir.AxisListType
np_bf16 = ml_dtypes.bfloat16

ND = 3                     # number of D chunks
HB = 2                     # batch items per big PSUM tile (2 banks)
LH = HB * L1               # 1024
D_TILES = [(0, 128), (128, 128), (256, 44)]
DROWS = [r for _, r in D_TILES]
C_TILES = [(0, 128), (128, 128)]
E_TILES = [(0, 128), (128, 128), (256, 128), (384, 128), (512, 44)]
NLC = L1 // 128

# ---- packed weight layouts: (name, rows-per-tile, cols) ----
PACKA = [("ident", [128], 128), ("w3", DROWS, K * C), ("w2t", [128, 128], E),
         ("w1t", DROWS, E)]
PACKB = [("w1c", DROWS, K * C), ("w2ctx", DROWS, K * C),
         ("w2att", [128, 128], K * C), ("hwt", [128, 128], C),
         ("hwgt", [128, 128], C), ("outwt", [128, 128], NCLS)]


def _pack_offsets(spec):
    offs, col = {}, 0
    for name, rows_list, cols in spec:
        lst = []
        for rows in rows_list:
            lst.append((rows, col, cols))
            col += cols
        offs[name] = lst
    return offs, col


A_OFF, WA = _pack_offsets(PACKA)
B_OFF, WB = _pack_offsets(PACKB)
# f32 pack cols: v2 per et at 2*et (rows esz); biases ct at 10+5*ct;
# outb on row 0 at col 20 + 4*b + c
WF = 36

_NC_CACHE = {}


def build_nc(stage=None, repeat=1):
    # default SWDGE ring (1024 descriptors) throttles the merged embedding
    # gathers (2112 descriptors outstanding); widen it.
    nc = bacc.Bacc("TRN2", target_bir_lowering=False, debug=False,
                   dynamic_dma_scratch_size=40960)

    d_ctx_ids = nc.dram_tensor("ctx_ids", [NL, 1], i32, kind="ExternalInput")
    d_asp_ids = nc.dram_tensor("asp_ids", [NM, 1], i32, kind="ExternalInput")
    d_emb = nc.dram_tensor("wordemb", [VOCAB, D], bf16, kind="ExternalInput")
    d_wpa = nc.dram_tensor("wpa", [128, WA], bf16, kind="ExternalInput")
    d_wpb = nc.dram_tensor("wpb", [128, WB], bf16, kind="ExternalInput")
    d_fpk = nc.dram_tensor("fpk", [128, WF], f32, kind="ExternalInput")
    d_out = nc.dram_tensor("out", [BL, NCLS], f32, kind="ExternalOutput")

    with tile.TileContext(nc) as tc:
        for _rep in range(repeat):
            _body(nc, tc, d_ctx_ids, d_asp_ids, d_emb, d_wpa, d_wpb, d_fpk,
                  d_out, stage=stage)
    nc.compile()
    return nc


def _body(nc, tc, d_ctx_ids, d_asp_ids, d_emb, d_wpa, d_wpb, d_fpk, d_out,
          stage=None):
    stack = contextlib.ExitStack()
    cst = stack.enter_context(tc.tile_pool(name="cst", bufs=1))
    per = stack.enter_context(tc.tile_pool(name="per", bufs=1))
    wk = stack.enter_context(tc.tile_pool(name="wk", bufs=3))
    ps2 = stack.enter_context(tc.tile_pool(name="ps2", bufs=2, space="PSUM"))
    pbig = stack.enter_context(tc.tile_pool(name="pbig", bufs=3, space="PSUM"))

    def finish(src):
        osb = wk.tile([BL, NCLS], f32, tag="osb", name="osb")
        nc.vector.tensor_copy(osb[:], src)
        nc.sync.dma_start(d_out.ap(), osb[:])
        stack.close()

    # ---- token ids + packed weights ----
    # DMA-bus arrival order is what matters (single FIFO bus in practice):
    # ids (Act queue) -> asp/ctx gathers (Pool) interleaved with wpa chunks
    # (SP queue) -> wpb (issued from Pool, behind the gathers).
    idxa = wk.tile([NM, 1], i32, tag="idxa", name="idxa")
    nc.sync.dma_start(idxa[:], d_asp_ids.ap())
    idx16 = wk.tile([128, NL // 128], i32, tag="idx16", name="idx16")
    nc.sync.dma_start(
        idx16[:], d_ctx_ids.ap().rearrange("(t p) o -> p (t o)", p=128))
    wpa = cst.tile([128, WA], bf16, tag="wpa", name="wpa")
    fpk = cst.tile([128, WF], f32, tag="fpk", name="fpk")
    wpb = cst.tile([128, WB], bf16, tag="wpb", name="wpb")

    def wpa_chunk(eng, names):
        lo = A_OFF[names[0]][0][1]
        last = A_OFF[names[-1]][-1]
        hi = last[1] + last[2]
        eng.dma_start(wpa[:, lo:hi], d_wpa.ap()[:, lo:hi])

    def wpb_chunk(eng, names):
        lo = B_OFF[names[0]][0][1]
        last = B_OFF[names[-1]][-1]
        hi = last[1] + last[2]
        eng.dma_start(wpb[:, lo:hi], d_wpb.ap()[:, lo:hi])

    wpa_chunk(nc.sync, ["ident"])
    nc.sync.dma_start(fpk[:], d_fpk.ap())
    wpa_chunk(nc.sync, ["w3"])
    wpa_chunk(nc.sync, ["w2t"])

    def va(name, it):
        rows, c0, w = A_OFF[name][it]
        return wpa[0:rows, c0:c0 + w]

    def vb(name, it):
        rows, c0, w = B_OFF[name][it]
        return wpb[0:rows, c0:c0 + w]

    def v2v(et):
        e0, esz = E_TILES[et]
        return fpk[0:esz, 2 * et:2 * et + 2]

    def biasv(ct, i):
        c0, csz = C_TILES[ct]
        return fpk[0:csz, 10 + 5 * ct + i:10 + 5 * ct + i + 1]

    ident = va("ident", 0)

    # ---- persistent activations (dense layouts) ----
    # ctxT: [128, dt*2048 + b*512 + l]; aspT: [128, dt*128 + b*16 + m]
    # (D chunk 2 has 44 valid rows; rows 44:128 hold transpose garbage)
    ctxT = per.tile([128, ND * NL], bf16, tag="ctxT", name="ctxT")
    aspT = per.tile([128, ND * 128], bf16, tag="aspT", name="aspT")
    attT = [per.tile([csz, NL], bf16, tag=f"attT{ct}", name=f"attT{ct}")
            for ct, (c0, csz) in enumerate(C_TILES)]
    UT = [per.tile([esz, NL], bf16, tag=f"UT{et}", name=f"UT{et}")
          for et, (e0, esz) in enumerate(E_TILES)]
    tyT = [per.tile([esz, NM], bf16, tag=f"tyT{et}", name=f"tyT{et}")
           for et, (e0, esz) in enumerate(E_TILES)]
    aT = [per.tile([csz, NM], bf16, tag=f"aT{ct}", name=f"aT{ct}")
          for ct, (c0, csz) in enumerate(C_TILES)]
    a_b = [per.tile([L2, C], bf16, tag=f"a_b{b}", name=f"a_b{b}")
           for b in range(BL)]
    alphaT = per.tile([L2, NL], bf16, tag="alphaT", name="alphaT")
    s1T = [per.tile([csz, NL], bf16, tag=f"s1T{ct}", name=f"s1T{ct}")
           for ct, (c0, csz) in enumerate(C_TILES)]
    mT = [per.tile([csz, NL], bf16, tag=f"mT{ct}", name=f"mT{ct}")
          for ct, (c0, csz) in enumerate(C_TILES)]
    pooled = [[per.tile([csz, 1], bf16, tag=f"pl{ct}_{b}", name=f"pl{ct}_{b}")
               for b in range(BL)] for ct, (c0, csz) in enumerate(C_TILES)]

    if stage == 0:
        return finish(ident[0:BL, 0:NCLS])

    def ctx_mv(dt, h):
        """[rows_dt, 1024] dense view of ctxT for b-pair h, D-chunk dt."""
        base = dt * NL + h * LH
        return ctxT[0:DROWS[dt], base:base + LH]

    # shifted-tap conv helper: accumulate K taps x n input chunks into the
    # per-b 512-col bank regions of a [rows, LH] PSUM tile — no halo
    # columns (matmul out must stay within one PSUM bank, <=512 f32 cols).
    def conv_taps(pv, rhs2, wcols, first, last):
        """pv: psum [rows, LH]; rhs2: list over chunks of [p, LH] dense pair
        views; wcols(dt, k) -> stationary AP.  first/last: accum flags."""
        n = len(rhs2)
        for j in range(HB):
            o = j * L1
            for dt in range(n):
                nc.tensor.matmul(pv[:, o:o + L1], wcols(dt, 1),
                                 rhs2[dt][:, o:o + L1],
                                 start=(first and dt == 0), stop=False)
            for dt in range(n):
                nc.tensor.matmul(pv[:, o + 1:o + L1], wcols(dt, 0),
                                 rhs2[dt][:, o:o + L1 - 1],
                                 start=False, stop=False)
            for dt in range(n):
                nc.tensor.matmul(pv[:, o:o + L1 - 1], wcols(dt, 2),
                                 rhs2[dt][:, o + 1:o + L1],
                                 start=False, stop=(last and dt == n - 1))

    # ---- gathers: one per batch item (SWDGE fixed cost ~1us dominates) ----
    # Pool queue interleave: the DGE drains each DMA through its transfer
    # (FIFO bus), so weight chunks ride between the gathers they must not
    # delay.  Indirect gathers only support a [<=128, 1] offset column.
    gab = wk.tile([NM, D], bf16, tag="gathab", name="gathab")
    nc.gpsimd.indirect_dma_start(
        out=gab[:], out_offset=None, in_=d_emb.ap(),
        in_offset=IndirectOffsetOnAxis(ap=idxa[:, 0:1], axis=0))
    gctx = []
    wchunks = {4: ("a", ["w1t"]), 7: ("b", ["w1c", "w2ctx"]),
               15: ("b", ["w2att", "hwt", "hwgt", "outwt"])}
    for t in range(NL // 128):
        gb = per.tile([128, D], bf16, tag=f"gb_{t}", name=f"gb_{t}")
        nc.gpsimd.indirect_dma_start(
            out=gb[:], out_offset=None, in_=d_emb.ap(),
            in_offset=IndirectOffsetOnAxis(ap=idx16[:, t:t + 1], axis=0))
        gctx.append(gb)
        if t in wchunks:
            which, names = wchunks[t]
            (wpa_chunk if which == "a" else wpb_chunk)(nc.gpsimd, names)

    trba = ps2.tile([128, ND * 128], bf16, tag="sm", name="trba")
    for dt, (d0, dsz) in enumerate(D_TILES):
        nc.tensor.transpose(out=trba[0:dsz, dt * 128:dt * 128 + NM],
                            in_=gab[:, d0:d0 + dsz],
                            identity=ident[:NM, :NM])
    nc.vector.tensor_copy(
        aspT[:].rearrange("p (z w) -> p z w", w=128)[:, :, 0:NM],
        trba[:].rearrange("p (z w) -> p z w", w=128)[:, :, 0:NM])

    def ctx_tile(t):
        gb = gctx[t]
        trb = ps2.tile([128, ND * 128], bf16, tag="sm", name="trb")
        for dt, (d0, dsz) in enumerate(D_TILES):
            nc.tensor.transpose(out=trb[0:dsz, dt * 128:(dt + 1) * 128],
                                in_=gb[:, d0:d0 + dsz],
                                identity=ident[:])
        dst = ctxT[:].rearrange("p (dt w) -> p dt w", w=NL)[
            :, :, t * 128:(t + 1) * 128]
        src = trb[:].rearrange("p (dt w) -> p dt w", w=128)
        if t % 2 == 0:
            nc.vector.tensor_copy(dst, src)
        else:
            nc.scalar.copy(dst, src)

    for t in range(4):
        ctx_tile(t)

    # ---- conv3 + relu -> aT (shifted taps over m within each b) ----
    for ct, (c0, csz) in enumerate(C_TILES):
        pa = ps2.tile([128, NM], f32, tag="sm", name="pa")
        pav = pa[:csz, :].rearrange("p (z w) -> p z w", w=L2)
        for dt in range(ND):
            rhs = aspT[0:DROWS[dt], dt * 128:dt * 128 + NM]
            nc.tensor.matmul(pa[:csz, :], va("w3", dt)[:, C + c0:C + c0 + csz],
                             rhs, start=(dt == 0), stop=False)
        for dt in range(ND):
            r3 = aspT[0:DROWS[dt], dt * 128:dt * 128 + NM].rearrange(
                "p (z w) -> p z w", w=L2)
            nc.tensor.matmul(pav[:, :, 1:L2], va("w3", dt)[:, c0:c0 + csz],
                             r3[:, :, 0:L2 - 1], start=False, stop=False)
        for dt in range(ND):
            r3 = aspT[0:DROWS[dt], dt * 128:dt * 128 + NM].rearrange(
                "p (z w) -> p z w", w=L2)
            nc.tensor.matmul(pav[:, :, 0:L2 - 1],
                             va("w3", dt)[:, 2 * C + c0:2 * C + c0 + csz],
                             r3[:, :, 1:L2], start=False, stop=(dt == ND - 1))
        nc.scalar.activation(aT[ct][:], pa[:csz, :], AF.Relu,
                             bias=biasv(ct, 0))

    for t in range(4, 8):
        ctx_tile(t)

    # a_b: per-batch [m, c]
    for b in range(BL):
        tr = ps2.tile([128, C], bf16, tag="sm", name="tr")
        for ct, (c0, csz) in enumerate(C_TILES):
            nc.tensor.transpose(out=tr[:L2, c0:c0 + csz],
                                in_=aT[ct][:, b * L2:(b + 1) * L2],
                                identity=ident[:csz, :csz])
        nc.vector.tensor_copy(a_b[b][:], tr[:L2, :])

    # aw -> ty, -ty^2
    for et, (e0, esz) in enumerate(E_TILES):
        paw = ps2.tile([128, NM], f32, tag="sm", name="paw")
        for ct, (c0, csz) in enumerate(C_TILES):
            nc.tensor.matmul(paw[:esz, :], va("w2t", ct)[:, e0:e0 + esz],
                             aT[ct][:], start=(ct == 0),
                             stop=(ct == len(C_TILES) - 1))
        nc.scalar.activation(tyT[et][:], paw[:esz, :], AF.Tanh)

    for t in range(8, 16):
        ctx_tile(t)

    if stage == 1:
        return finish(ctxT[0:BL, 0:NCLS])

    # ---- cw -> tx -> U, U2 (interleaved with conv1) ----
    def cw_unit(h, et):
        e0, esz = E_TILES[et]
        pcw = pbig.tile([128, LH], f32, tag="big", name="pcw")
        for j in range(HB):
            o = j * L1
            for dt in range(ND):
                nc.tensor.matmul(pcw[:esz, o:o + L1],
                                 va("w1t", dt)[:, e0:e0 + esz],
                                 ctx_mv(dt, h)[:, o:o + L1], start=(dt == 0),
                                 stop=(dt == ND - 1))
        hs = slice(h * LH, (h + 1) * LH)
        tx = wk.tile([128, LH], bf16, tag="tx", name="tx")
        nc.scalar.activation(tx[:esz, :], pcw[:esz, :], AF.Tanh)
        sq = wk.tile([128, LH], bf16, tag="sq", name="sq")
        nc.vector.tensor_tensor(sq[:esz, :], tx[:esz, :], tx[:esz, :],
                                op=ALU.mult)
        nc.vector.tensor_scalar(UT[et][:, hs], sq[:esz, :], v2v(et)[:, 1:2],
                                v2v(et)[:, 0:1], op0=ALU.mult, op1=ALU.add)

    def conv1_unit(h, ct):
        c0, csz = C_TILES[ct]
        ps1 = pbig.tile([128, LH], f32, tag="big", name="ps1")
        conv_taps(ps1[:csz, :], [ctx_mv(dt, h) for dt in range(ND)],
                  lambda dt, k: vb("w1c", dt)[:, k * C + c0:k * C + c0 + csz],
                  True, True)
        nc.scalar.activation(s1T[ct][:, h * LH:(h + 1) * LH], ps1[:csz, :],
                             AF.Tanh, bias=biasv(ct, 1))

    for et in range(len(E_TILES)):
        cw_unit(0, et)
    conv1_unit(0, 0)
    if stage == 31:
        return finish(s1T[0][0:BL, 0:NCLS])
    for et in range(len(E_TILES)):
        cw_unit(1, et)
    conv1_unit(0, 1)

    if stage == 3:
        return finish(UT[0][0:BL, 0:NCLS])

    # ---- score -> softmax -> alphaT ----
    def score_unit(b):
        psc = ps2.tile([128, NLC * L2], f32, tag="sm", name="sc")
        n_et = len(E_TILES)
        for lc in range(NLC):
            col = b * L1 + lc * 128
            reg = psc[:, lc * L2:(lc + 1) * L2]
            for et, (e0, esz) in enumerate(E_TILES):
                nc.tensor.matmul(reg, UT[et][:esz, col:col + 128],
                                 tyT[et][:, b * L2:(b + 1) * L2],
                                 start=(et == 0), stop=(et == n_et - 1))
        al_u = wk.tile([128, NLC * L2], bf16, tag="alu", name="alu")
        nc.scalar.activation(al_u[:], psc[:], AF.Exp)
        rs4 = wk.tile([128, NLC], f32, tag="rs4", name="rs4")
        nc.vector.reduce_sum(
            out=rs4[:], in_=al_u[:].rearrange("p (z m) -> p z m", m=L2),
            axis=AX.X)
        rc4 = wk.tile([128, NLC], f32, tag="rc4", name="rc4")
        nc.vector.reciprocal(rc4[:], rs4[:])
        trb4 = ps2.tile([128, L1], bf16, tag="sm", name="trb4")
        for lc in range(NLC):
            al = wk.tile([128, L2], bf16, tag=f"al{lc}", name=f"al{lc}")
            nc.vector.tensor_scalar_mul(al[:], al_u[:, lc * L2:(lc + 1) * L2],
                                        rc4[:, lc:lc + 1])
            nc.tensor.transpose(out=trb4[:L2, lc * 128:(lc + 1) * 128],
                                in_=al[:], identity=ident[:])
        nc.scalar.copy(alphaT[:, b * L1:(b + 1) * L1], trb4[:L2, :])

    def att_unit(h):
        for ct, (c0, csz) in enumerate(C_TILES):
            pat = pbig.tile([128, LH], f32, tag="big", name="pat")
            for j in range(HB):
                b = h * HB + j
                nc.tensor.matmul(pat[:csz, j * L1:(j + 1) * L1],
                                 a_b[b][:, c0:c0 + csz],
                                 alphaT[:, b * L1:(b + 1) * L1],
                                 start=True, stop=True)
            if ct == 0:
                nc.scalar.copy(attT[ct][:, h * LH:(h + 1) * LH], pat[:csz, :])
            else:
                nc.vector.tensor_copy(attT[ct][:, h * LH:(h + 1) * LH],
                                      pat[:csz, :])

    score_unit(0)
    score_unit(1)
    conv1_unit(1, 0)
    att_unit(0)
    if stage == 41:
        return finish(attT[0][0:BL, 0:NCLS])
    conv1_unit(1, 1)
    score_unit(2)
    score_unit(3)

    if stage == 4:
        return finish(alphaT[0:BL, 0:NCLS])

    # ---- conv2 (relu, asp folded) -> m ----
    def conv2_unit(h, ct):
        c0, csz = C_TILES[ct]
        pg = pbig.tile([128, LH], f32, tag="big", name="pg")
        conv_taps(pg[:csz, :], [ctx_mv(dt, h) for dt in range(ND)],
                  lambda dt, k: vb("w2ctx", dt)[:, k * C + c0:k * C + c0 + csz],
                  True, False)
        conv_taps(pg[:csz, :],
                  [attT[jt][:, h * LH:(h + 1) * LH] for jt in range(2)],
                  lambda jt, k: vb("w2att", jt)[:, k * C + c0:k * C + c0 + csz],
                  False, True)
        gg = wk.tile([128, LH], bf16, tag="gg", name="gg")
        nc.scalar.activation(gg[:csz, :], pg[:csz, :], AF.Relu,
                             bias=biasv(ct, 2))
        hs = slice(h * LH, (h + 1) * LH)
        nc.vector.tensor_tensor(mT[ct][:, hs], s1T[ct][:, hs], gg[:csz, :],
                                op=ALU.mult)

    # conv2(h0) only needs attT h0 — emit before att_unit(1) so PE's
    # in-order queue isn't head-of-line blocked on softmax b2/b3.
    conv2_unit(0, 0)
    att_unit(1)
    conv2_unit(0, 1)

    if stage == 5:
        return finish(mT[0][0:BL, 0:NCLS])

    # ---- highway + maxpool ----
    def hw_unit(h, ct):
        c0, csz = C_TILES[ct]
        hs = slice(h * LH, (h + 1) * LH)
        ph = pbig.tile([128, LH], f32, tag="big", name="ph")
        for j in range(HB):
            o = h * LH + j * L1
            for jt, (j0, jsz) in enumerate(C_TILES):
                nc.tensor.matmul(ph[:csz, j * L1:(j + 1) * L1],
                                 vb("hwt", jt)[:, c0:c0 + csz],
                                 mT[jt][:, o:o + L1], start=(jt == 0),
                                 stop=(jt == len(C_TILES) - 1))
        hh = wk.tile([128, LH], bf16, tag="hh", name="hh")
        nc.scalar.activation(hh[:csz, :], ph[:csz, :], AF.Relu,
                             bias=biasv(ct, 3))
        phg = pbig.tile([128, LH], f32, tag="big", name="phg")
        for j in range(HB):
            o = h * LH + j * L1
            for jt, (j0, jsz) in enumerate(C_TILES):
                nc.tensor.matmul(phg[:csz, j * L1:(j + 1) * L1],
                                 vb("hwgt", jt)[:, c0:c0 + csz],
                                 mT[jt][:, o:o + L1], start=(jt == 0),
                                 stop=(jt == len(C_TILES) - 1))
        gt = wk.tile([128, LH], bf16, tag="gt", name="gt")
        nc.scalar.activation(gt[:csz, :], phg[:csz, :], AF.Sigmoid,
                             bias=biasv(ct, 4))
        dd = wk.tile([128, LH], bf16, tag="dd", name="dd")
        nc.vector.tensor_tensor(dd[:csz, :], hh[:csz, :], mT[ct][:, hs],
                                op=ALU.subtract)
        ee = wk.tile([128, LH], bf16, tag="ee", name="ee")
        nc.vector.tensor_tensor(ee[:csz, :], gt[:csz, :], dd[:csz, :],
                                op=ALU.mult)
        m2 = wk.tile([128, LH], bf16, tag="m2", name="m2")
        nc.vector.tensor_tensor(m2[:csz, :], ee[:csz, :], mT[ct][:, hs],
                                op=ALU.add)
        for j in range(HB):
            b = h * HB + j
            nc.vector.reduce_max(
                out=pooled[ct][b][:],
                in_=m2[:csz, j * L1:(j + 1) * L1], axis=AX.X)

    po = ps2.tile([128, L2], f32, tag="sm", name="po")

    def classifier(b):
        for ct in range(len(C_TILES)):
            nc.tensor.matmul(po[0:1, b * 4:b * 4 + NCLS],
                             pooled[ct][b][:], vb("outwt", ct)[:, 0:NCLS],
                             start=(ct == 0), stop=(ct == len(C_TILES) - 1))

    hw_unit(0, 0)
    if stage == 6:
        return finish(mT[0][0:BL, 0:NCLS])
    conv2_unit(1, 0)
    hw_unit(0, 1)
    conv2_unit(1, 1)
    if stage == 7:
        return finish(mT[0][0:BL, 0:NCLS])
    for b in range(HB):
        classifier(b)
    if stage == 8:
        return finish(mT[0][0:BL, 0:NCLS])
    hw_unit(1, 0)
    hw_unit(1, 1)
    for b in range(HB, BL):
        classifier(b)
    if stage == 9:
        return finish(mT[0][0:BL, 0:NCLS])

    # out[b, c] = po[0, b*4+c] + out_b[c]
    osb = wk.tile([1, BL * NCLS], f32, tag="osb", name="osb")
    v3 = lambda ap, o: ap.rearrange("p (b x) -> p b x", x=4)[:, :, o:o + NCLS]
    nc.vector.tensor_tensor(
        osb[:].rearrange("p (b x) -> p b x", x=NCLS),
        v3(po[0:1, 0:BL * 4], 0), v3(fpk[0:1, 20:20 + BL * 4], 0), op=ALU.add)
    nc.sync.dma_start(
        d_out.ap().rearrange("(o b) c -> o (b c)", o=1), osb[:])
    stack.close()


def prep_inputs(context_ids, aspect_ids, wordemb, conv3_w, conv3_b, conv1_w,
                conv1_b, conv2_w, conv2_b, attn_W, attn_V, asp_w, asp_b, hw_w,
                hw_b, hwg_w, hwg_b, out_w, out_b):
    """Host-side prep: weight layout transforms + bf16 casts (weights only)."""
    f = np.float32
    attn_W = np.asarray(attn_W, f)
    w2 = np.asarray(conv2_w, f)
    asp_w = np.asarray(asp_w, f)

    mats = {
        "ident": np.eye(128, dtype=f),
        "w3": np.asarray(conv3_w, f).transpose(1, 2, 0).reshape(D, K * C),
        "w2t": np.ascontiguousarray(attn_W[:, D:].T),
        "w1t": np.ascontiguousarray(attn_W[:, :D].T),
        "w1c": np.asarray(conv1_w, f).transpose(1, 2, 0).reshape(D, K * C),
        "w2ctx": w2[:, :D, :].transpose(1, 2, 0).reshape(D, K * C),
        "w2att": np.einsum("aok,oc->ack", w2[:, D:, :], asp_w)
                .transpose(1, 2, 0).reshape(C, K * C),
        "hwt": np.ascontiguousarray(np.asarray(hw_w, f).T),
        "hwgt": np.ascontiguousarray(np.asarray(hwg_w, f).T),
        "outwt": np.ascontiguousarray(np.asarray(out_w, f).T),
    }
    def build_pack(offs, width):
        pk = np.zeros((128, width), np_bf16)
        for name, lst in offs.items():
            m = mats[name]
            r0 = 0
            for rows, c0, w in lst:
                pk[0:rows, c0:c0 + w] = m[r0:r0 + rows].astype(np_bf16)
                r0 += rows
        return pk

    fpk = np.zeros((128, WF), f)
    V = np.asarray(attn_V, f)[0]
    for et, (e0, esz) in enumerate(E_TILES):
        fpk[0:esz, 2 * et] = V[e0:e0 + esz]
        fpk[0:esz, 2 * et + 1] = -V[e0:e0 + esz]
    biases = np.stack([
        np.asarray(conv3_b, f),
        np.asarray(conv1_b, f),
        np.asarray(conv2_b, f) + np.einsum("aok,o->a", w2[:, D:, :],
                                           np.asarray(asp_b, f)),
        np.asarray(hw_b, f),
        np.asarray(hwg_b, f)], axis=1)
    for ct, (c0, csz) in enumerate(C_TILES):
        fpk[0:csz, 10 + 5 * ct:15 + 5 * ct] = biases[c0:c0 + csz]
    for b in range(BL):
        fpk[0, 20 + 4 * b:20 + 4 * b + NCLS] = np.asarray(out_b, f)

    shared = {
        "wordemb": np.asarray(wordemb, f).astype(np_bf16),
        "wpa": build_pack(A_OFF, WA),
        "wpb": build_pack(B_OFF, WB),
        "fpk": fpk,
    }
    in_maps = []
    for c in range(NCORES):
        m = dict(shared)
        m["ctx_ids"] = np.ascontiguousarray(
            np.asarray(context_ids, np.int32)[c * BL:(c + 1) * BL]
        ).reshape(NL, 1)
        m["asp_ids"] = np.ascontiguousarray(
            np.asarray(aspect_ids, np.int32)[c * BL:(c + 1) * BL]
        ).reshape(NM, 1)
        in_maps.append(m)
    return in_maps


def kernel(**inputs):
    if "nc" not in _NC_CACHE:
        _NC_CACHE["nc"] = build_nc()
    nc = _NC_CACHE["nc"]
    in_maps = prep_inputs(**inputs)
    res = run_bass_kernel_spmd(nc, in_maps, core_ids=list(range(NCORES)))
    return np.concatenate([res.results[c]["out"] for c in range(NCORES)], axis=0)


if __name__ == "__main__":
    print("building...")
    nc = build_nc()
    print("built ok")


# revision 78
# speedup vs baseline: 1.1591x; 1.0128x over previous
"""Trainium2 Bass kernel for the GCM aspect-sentiment model.

Sharding: pure data parallelism — batch (32) split across 8 NeuronCores
(4 items/core); embedding table + all weights replicated.

Per-core plan (all matmuls bf16 with fp32 PSUM accumulation):
  - embedding rows gathered from DRAM via indirect DMA (one 128-row
    gather per token tile; [<=128, 1] offset columns only), cast bf16,
    PE-transposed to channel-major dense [D, B*L] layout.
  - attention via 1st-order expansion of tanh(cw+aw) in the small aspect
    term: score = (V(1-tx^2)).ty; the l-constant term V.tx cancels in
    softmax and the ty^2 term is below bf16 noise (verified on host).
  - convs as shifted-tap matmuls: tap k accumulates into a shifted
    column range of the same PSUM bank, so zero-padding needs no halo
    columns or memsets, and activation layouts are fully dense.
  - weights land via packed [128, W] tensors; chunks are issued on the
    SP queue (early consumers) or interleaved into the Pool gather
    stream (the Pool DGE drains each DMA through its transfer on a FIFO
    bus, so ordering is placement).
  - conv1 and conv2(h0) interleaved into the attention phase (PE
    otherwise idles there while Act/DVE chew tanh/softmax chains, and
    idle PE drops to a lower p-state clock).
  - highway + per-b maxpool + classifier batched into one PSUM tile,
    single bias add, single output DMA.

Constraints learned on HW:
  - every vector/scalar-engine SBUF write lands on a 256B-aligned column
    (2B-aligned writes take a ~70us slow path);
  - matmul out/moving free size <= 512 (one PSUM bank), out base
    partition in {0, 32, 64};
  - tensor_tensor_reduce with op1=max hangs the device; gpsimd cannot
    touch PSUM; fp8 on conv1/conv2 breaks the 2e-2 gate (host-verified).
"""

import contextlib

import numpy as np
import ml_dtypes

import concourse.bacc as bacc
import concourse.mybir as mybir
import concourse.tile as tile
from concourse.bass import IndirectOffsetOnAxis
from concourse.bass_utils import run_bass_kernel_spmd

B, L1, L2 = 32, 512, 16
D, C, NCLS = 300, 256, 3
K = 3
VOCAB = 50000
E = D + C
NCORES = 8
BL = B // NCORES          # batch per core
NL = BL * L1              # 2048 context tokens per core
NM = BL * L2              # 64 aspect tokens per core

bf16 = mybir.dt.bfloat16
f32 = mybir.dt.float32
i32 = mybir.dt.int32
AF = mybir.ActivationFunctionType
ALU = mybir.AluOpType
AX = mybir.AxisListType
np_bf16 = ml_dtypes.bfloat16

ND = 3                     # number of D chunks
HB = 2                     # batch items per big PSUM tile (2 banks)
LH = HB * L1               # 1024
D_TILES = [(0, 128), (128, 128), (256, 44)]
DROWS = [r for _, r in D_TILES]
C_TILES = [(0, 128), (128, 128)]
E_TILES = [(0, 128), (128, 128), (256, 128), (384, 128), (512, 44)]
NLC = L1 // 128

# ---- packed weight layouts: (name, rows-per-tile, cols) ----
PACKA = [("ident", [128], 128), ("w3", DROWS, K * C), ("w2t", [128, 128], E),
         ("w1t", DROWS, E)]
PACKB = [("w1c", DROWS, K * C), ("w2ctx", DROWS, K * C),
         ("w2att", [128, 128], K * C), ("hwt", [128, 128], C),
         ("hwgt", [128, 128], C), ("outwt", [128, 128], NCLS)]


def _pack_offsets(spec):
    offs, col = {}, 0
    for name, rows_list, cols in spec:
        lst = []
        for rows in rows_list:
            lst.append((rows, col, cols))
            col += cols
        offs[name] = lst
    return offs, col


A_OFF, WA = _pack_offsets(PACKA)
B_OFF, WB = _pack_offsets(PACKB)
# f32 pack cols: v2 per et at 2*et (rows esz); biases ct at 10+5*ct;
# outb on row 0 at col 20 + 4*b + c
WF = 36

_NC_CACHE = {}


def build_nc(stage=None, repeat=1):
    # default SWDGE ring (1024 descriptors) throttles the merged embedding
    # gathers (2112 descriptors outstanding); widen it.
    nc = bacc.Bacc("TRN2", target_bir_lowering=False, debug=False,
                   dynamic_dma_scratch_size=40960)

    d_ctx_ids = nc.dram_tensor("ctx_ids", [NL, 1], i32, kind="ExternalInput")
    d_asp_ids = nc.dram_tensor("asp_ids", [NM, 1], i32, kind="ExternalInput")
    d_emb = nc.dram_tensor("wordemb", [VOCAB, D], bf16, kind="ExternalInput")
    d_wpa = nc.dram_tensor("wpa", [128, WA], bf16, kind="ExternalInput")
    d_wpb = nc.dram_tensor("wpb", [128, WB], bf16, kind="ExternalInput")
    d_fpk = nc.dram_tensor("fpk", [128, WF], f32, kind="ExternalInput")
    d_out = nc.dram_tensor("out", [BL, NCLS], f32, kind="ExternalOutput")

    with tile.TileContext(nc) as tc:
        for _rep in range(repeat):
            _body(nc, tc, d_ctx_ids, d_asp_ids, d_emb, d_wpa, d_wpb, d_fpk,
                  d_out, stage=stage)
    nc.compile()
    return nc


def _body(nc, tc, d_ctx_ids, d_asp_ids, d_emb, d_wpa, d_wpb, d_fpk, d_out,
          stage=None):
    stack = contextlib.ExitStack()
    cst = stack.enter_context(tc.tile_pool(name="cst", bufs=1))
    per = stack.enter_context(tc.tile_pool(name="per", bufs=1))
    wk = stack.enter_context(tc.tile_pool(name="wk", bufs=3))
    ps2 = stack.enter_context(tc.tile_pool(name="ps2", bufs=2, space="PSUM"))
    pbig = stack.enter_context(tc.tile_pool(name="pbig", bufs=3, space="PSUM"))

    def finish(src):
        osb = wk.tile([BL, NCLS], f32, tag="osb", name="osb")
        nc.vector.tensor_copy(osb[:], src)
        nc.sync.dma_start(d_out.ap(), osb[:])
        stack.close()

    # ---- token ids + packed weights ----
    # DMA-bus arrival order is what matters (single FIFO bus in practice):
    # ids (Act queue) -> asp/ctx gathers (Pool) interleaved with wpa chunks
    # (SP queue) -> wpb (issued from Pool, behind the gathers).
    wpa = cst.tile([128, WA], bf16, tag="wpa", name="wpa")
    fpk = cst.tile([128, WF], f32, tag="fpk", name="fpk")
    wpb = cst.tile([128, WB], bf16, tag="wpb", name="wpb")
    idxa = wk.tile([NM, 1], i32, tag="idxa", name="idxa")
    idx16 = wk.tile([128, NL // 128], i32, tag="idx16", name="idx16")

    def wpa_chunk(eng, names):
        lo = A_OFF[names[0]][0][1]
        last = A_OFF[names[-1]][-1]
        hi = last[1] + last[2]
        eng.dma_start(wpa[:, lo:hi], d_wpa.ap()[:, lo:hi])

    def wpb_chunk(eng, names):
        lo = B_OFF[names[0]][0][1]
        last = B_OFF[names[-1]][-1]
        hi = last[1] + last[2]
        eng.dma_start(wpb[:, lo:hi], d_wpb.ap()[:, lo:hi])

    nc.sync.dma_start(idxa[:], d_asp_ids.ap())
    nc.sync.dma_start(
        idx16[:], d_ctx_ids.ap().rearrange("(t p) o -> p (t o)", p=128))
    wpa_chunk(nc.sync, ["ident"])
    nc.sync.dma_start(fpk[:], d_fpk.ap())
    wpa_chunk(nc.sync, ["w3"])
    wpa_chunk(nc.sync, ["w2t"])

    def va(name, it):
        rows, c0, w = A_OFF[name][it]
        return wpa[0:rows, c0:c0 + w]

    def vb(name, it):
        rows, c0, w = B_OFF[name][it]
        return wpb[0:rows, c0:c0 + w]

    def v2v(et):
        e0, esz = E_TILES[et]
        return fpk[0:esz, 2 * et:2 * et + 2]

    def biasv(ct, i):
        c0, csz = C_TILES[ct]
        return fpk[0:csz, 10 + 5 * ct + i:10 + 5 * ct + i + 1]

    ident = va("ident", 0)

    # ---- persistent activations (dense layouts) ----
    # ctxT: [128, dt*2048 + b*512 + l]; aspT: [128, dt*128 + b*16 + m]
    # (D chunk 2 has 44 valid rows; rows 44:128 hold transpose garbage)
    ctxT = per.tile([128, ND * NL], bf16, tag="ctxT", name="ctxT")
    aspT = per.tile([128, ND * 128], bf16, tag="aspT", name="aspT")
    attT = [per.tile([csz, NL], bf16, tag=f"attT{ct}", name=f"attT{ct}")
            for ct, (c0, csz) in enumerate(C_TILES)]
    UT = [per.tile([esz, NL], bf16, tag=f"UT{et}", name=f"UT{et}")
          for et, (e0, esz) in enumerate(E_TILES)]
    tyT = [per.tile([esz, NM], bf16, tag=f"tyT{et}", name=f"tyT{et}")
           for et, (e0, esz) in enumerate(E_TILES)]
    aT = [per.tile([csz, NM], bf16, tag=f"aT{ct}", name=f"aT{ct}")
          for ct, (c0, csz) in enumerate(C_TILES)]
    a_b = [per.tile([L2, C], bf16, tag=f"a_b{b}", name=f"a_b{b}")
           for b in range(BL)]
    alphaT = per.tile([L2, NL], bf16, tag="alphaT", name="alphaT")
    s1T = [per.tile([csz, NL], bf16, tag=f"s1T{ct}", name=f"s1T{ct}")
           for ct, (c0, csz) in enumerate(C_TILES)]
    mT = [per.tile([csz, NL], bf16, tag=f"mT{ct}", name=f"mT{ct}")
          for ct, (c0, csz) in enumerate(C_TILES)]
    pooled = [[per.tile([csz, 1], bf16, tag=f"pl{ct}_{b}", name=f"pl{ct}_{b}")
               for b in range(BL)] for ct, (c0, csz) in enumerate(C_TILES)]

    if stage == 0:
        return finish(ident[0:BL, 0:NCLS])

    def ctx_mv(dt, h):
        """[rows_dt, 1024] dense view of ctxT for b-pair h, D-chunk dt."""
        base = dt * NL + h * LH
        return ctxT[0:DROWS[dt], base:base + LH]

    # shifted-tap conv helper: accumulate K taps x n input chunks into the
    # per-b 512-col bank regions of a [rows, LH] PSUM tile — no halo
    # columns (matmul out must stay within one PSUM bank, <=512 f32 cols).
    def conv_taps(pv, rhs2, wcols, first, last):
        """pv: psum [rows, LH]; rhs2: list over chunks of [p, LH] dense pair
        views; wcols(dt, k) -> stationary AP.  first/last: accum flags."""
        n = len(rhs2)
        for j in range(HB):
            o = j * L1
            for dt in range(n):
                nc.tensor.matmul(pv[:, o:o + L1], wcols(dt, 1),
                                 rhs2[dt][:, o:o + L1],
                                 start=(first and dt == 0), stop=False)
            for dt in range(n):
                nc.tensor.matmul(pv[:, o + 1:o + L1], wcols(dt, 0),
                                 rhs2[dt][:, o:o + L1 - 1],
                                 start=False, stop=False)
            for dt in range(n):
                nc.tensor.matmul(pv[:, o:o + L1 - 1], wcols(dt, 2),
                                 rhs2[dt][:, o + 1:o + L1],
                                 start=False, stop=(last and dt == n - 1))

    # ---- gathers: one per batch item (SWDGE fixed cost ~1us dominates) ----
    # Pool queue interleave: the DGE drains each DMA through its transfer
    # (FIFO bus), so weight chunks ride between the gathers they must not
    # delay.  Indirect gathers only support a [<=128, 1] offset column.
    gab = wk.tile([NM, D], bf16, tag="gathab", name="gathab")
    nc.gpsimd.indirect_dma_start(
        out=gab[:], out_offset=None, in_=d_emb.ap(),
        in_offset=IndirectOffsetOnAxis(ap=idxa[:, 0:1], axis=0))
    gctx = []
    wchunks = {4: ("a", ["w1t"]), 7: ("b", ["w1c", "w2ctx"]),
               15: ("b", ["w2att", "hwt", "hwgt", "outwt"])}
    for t in range(NL // 128):
        gb = per.tile([128, D], bf16, tag=f"gb_{t}", name=f"gb_{t}")
        nc.gpsimd.indirect_dma_start(
            out=gb[:], out_offset=None, in_=d_emb.ap(),
            in_offset=IndirectOffsetOnAxis(ap=idx16[:, t:t + 1], axis=0))
        gctx.append(gb)
        if t in wchunks:
            which, names = wchunks[t]
            (wpa_chunk if which == "a" else wpb_chunk)(nc.gpsimd, names)

    trba = ps2.tile([128, ND * 128], bf16, tag="sm", name="trba")
    for dt, (d0, dsz) in enumerate(D_TILES):
        nc.tensor.transpose(out=trba[0:dsz, dt * 128:dt * 128 + NM],
                            in_=gab[:, d0:d0 + dsz],
                            identity=ident[:NM, :NM])
    nc.vector.tensor_copy(
        aspT[:].rearrange("p (z w) -> p z w", w=128)[:, :, 0:NM],
        trba[:].rearrange("p (z w) -> p z w", w=128)[:, :, 0:NM])

    def ctx_tile(t):
        gb = gctx[t]
        trb = ps2.tile([128, ND * 128], bf16, tag="sm", name="trb")
        for dt, (d0, dsz) in enumerate(D_TILES):
            nc.tensor.transpose(out=trb[0:dsz, dt * 128:(dt + 1) * 128],
                                in_=gb[:, d0:d0 + dsz],
                                identity=ident[:])
        dst = ctxT[:].rearrange("p (dt w) -> p dt w", w=NL)[
            :, :, t * 128:(t + 1) * 128]
        src = trb[:].rearrange("p (dt w) -> p dt w", w=128)
        if t % 2 == 0:
            nc.vector.tensor_copy(dst, src)
        else:
            nc.scalar.copy(dst, src)

    for t in range(4):
        ctx_tile(t)

    # ---- conv3 + relu -> aT (shifted taps over m within each b) ----
    for ct, (c0, csz) in enumerate(C_TILES):
        pa = ps2.tile([128, NM], f32, tag="sm", name="pa")
        pav = pa[:csz, :].rearrange("p (z w) -> p z w", w=L2)
        for dt in range(ND):
            rhs = aspT[0:DROWS[dt], dt * 128:dt * 128 + NM]
            nc.tensor.matmul(pa[:csz, :], va("w3", dt)[:, C + c0:C + c0 + csz],
                             rhs, start=(dt == 0), stop=False)
        for dt in range(ND):
            r3 = aspT[0:DROWS[dt], dt * 128:dt * 128 + NM].rearrange(
                "p (z w) -> p z w", w=L2)
            nc.tensor.matmul(pav[:, :, 1:L2], va("w3", dt)[:, c0:c0 + csz],
                             r3[:, :, 0:L2 - 1], start=False, stop=False)
        for dt in range(ND):
            r3 = aspT[0:DROWS[dt], dt * 128:dt * 128 + NM].rearrange(
                "p (z w) -> p z w", w=L2)
            nc.tensor.matmul(pav[:, :, 0:L2 - 1],
                             va("w3", dt)[:, 2 * C + c0:2 * C + c0 + csz],
                             r3[:, :, 1:L2], start=False, stop=(dt == ND - 1))
        nc.scalar.activation(aT[ct][:], pa[:csz, :], AF.Relu,
                             bias=biasv(ct, 0))

    for t in range(4, 8):
        ctx_tile(t)

    # a_b: per-batch [m, c]
    for b in range(BL):
        tr = ps2.tile([128, C], bf16, tag="sm", name="tr")
        for ct, (c0, csz) in enumerate(C_TILES):
            nc.tensor.transpose(out=tr[:L2, c0:c0 + csz],
                                in_=aT[ct][:, b * L2:(b + 1) * L2],
                                identity=ident[:csz, :csz])
        nc.vector.tensor_copy(a_b[b][:], tr[:L2, :])

    # aw -> ty, -ty^2
    for et, (e0, esz) in enumerate(E_TILES):
        paw = ps2.tile([128, NM], f32, tag="sm", name="paw")
        for ct, (c0, csz) in enumerate(C_TILES):
            nc.tensor.matmul(paw[:esz, :], va("w2t", ct)[:, e0:e0 + esz],
                             aT[ct][:], start=(ct == 0),
                             stop=(ct == len(C_TILES) - 1))
        nc.scalar.activation(tyT[et][:], paw[:esz, :], AF.Tanh)

    for t in range(8, 16):
        ctx_tile(t)

    if stage == 1:
        return finish(ctxT[0:BL, 0:NCLS])

    # ---- cw -> tx -> U, U2 (interleaved with conv1) ----
    def cw_unit(h, et):
        e0, esz = E_TILES[et]
        pcw = pbig.tile([128, LH], f32, tag="big", name="pcw")
        for j in range(HB):
            o = j * L1
            for dt in range(ND):
                nc.tensor.matmul(pcw[:esz, o:o + L1],
                                 va("w1t", dt)[:, e0:e0 + esz],
                                 ctx_mv(dt, h)[:, o:o + L1], start=(dt == 0),
                                 stop=(dt == ND - 1))
        hs = slice(h * LH, (h + 1) * LH)
        tx = wk.tile([128, LH], bf16, tag="tx", name="tx")
        nc.scalar.activation(tx[:esz, :], pcw[:esz, :], AF.Tanh)
        sq = wk.tile([128, LH], bf16, tag="sq", name="sq")
        nc.vector.tensor_tensor(sq[:esz, :], tx[:esz, :], tx[:esz, :],
                                op=ALU.mult)
        nc.vector.tensor_scalar(UT[et][:, hs], sq[:esz, :], v2v(et)[:, 1:2],
                                v2v(et)[:, 0:1], op0=ALU.mult, op1=ALU.add)

    def conv1_unit(h, ct):
        c0, csz = C_TILES[ct]
        ps1 = pbig.tile([128, LH], f32, tag="big", name="ps1")
        conv_taps(ps1[:csz, :], [ctx_mv(dt, h) for dt in range(ND)],
                  lambda dt, k: vb("w1c", dt)[:, k * C + c0:k * C + c0 + csz],
                  True, True)
        nc.scalar.activation(s1T[ct][:, h * LH:(h + 1) * LH], ps1[:csz, :],
                             AF.Tanh, bias=biasv(ct, 1))

    for et in range(len(E_TILES)):
        cw_unit(0, et)
    conv1_unit(0, 0)
    if stage == 31:
        return finish(s1T[0][0:BL, 0:NCLS])
    for et in range(len(E_TILES)):
        cw_unit(1, et)
    conv1_unit(0, 1)

    if stage == 3:
        return finish(UT[0][0:BL, 0:NCLS])

    # ---- score -> softmax -> alphaT ----
    def score_unit(b):
        psc = ps2.tile([128, NLC * L2], f32, tag="sm", name="sc")
        n_et = len(E_TILES)
        for lc in range(NLC):
            col = b * L1 + lc * 128
            reg = psc[:, lc * L2:(lc + 1) * L2]
            for et, (e0, esz) in enumerate(E_TILES):
                nc.tensor.matmul(reg, UT[et][:esz, col:col + 128],
                                 tyT[et][:, b * L2:(b + 1) * L2],
                                 start=(et == 0), stop=(et == n_et - 1))
        al_u = wk.tile([128, NLC * L2], bf16, tag="alu", name="alu")
        nc.scalar.activation(al_u[:], psc[:], AF.Exp)
        rs4 = wk.tile([128, NLC], f32, tag="rs4", name="rs4")
        nc.vector.reduce_sum(
            out=rs4[:], in_=al_u[:].rearrange("p (z m) -> p z m", m=L2),
            axis=AX.X)
        rc4 = wk.tile([128, NLC], f32, tag="rc4", name="rc4")
        nc.vector.reciprocal(rc4[:], rs4[:])
        trb4 = ps2.tile([128, L1], bf16, tag="sm", name="trb4")
        for lc in range(NLC):
            al = wk.tile([128, L2], bf16, tag=f"al{lc}", name=f"al{lc}")
            nc.vector.tensor_scalar_mul(al[:], al_u[:, lc * L2:(lc + 1) * L2],
                                        rc4[:, lc:lc + 1])
            nc.tensor.transpose(out=trb4[:L2, lc * 128:(lc + 1) * 128],
                                in_=al[:], identity=ident[:])
        nc.scalar.copy(alphaT[:, b * L1:(b + 1) * L1], trb4[:L2, :])

    def att_unit(h):
        for ct, (c0, csz) in enumerate(C_TILES):
            pat = pbig.tile([128, LH], f32, tag="big", name="pat")
            for j in range(HB):
                b = h * HB + j
                nc.tensor.matmul(pat[:csz, j * L1:(j + 1) * L1],
                                 a_b[b][:, c0:c0 + csz],
                                 alphaT[:, b * L1:(b + 1) * L1],
                                 start=True, stop=True)
            if ct == 0:
                nc.scalar.copy(attT[ct][:, h * LH:(h + 1) * LH], pat[:csz, :])
            else:
                nc.vector.tensor_copy(attT[ct][:, h * LH:(h + 1) * LH],
                                      pat[:csz, :])

    score_unit(0)
    score_unit(1)
    conv1_unit(1, 0)
    att_unit(0)
    if stage == 41:
        return finish(attT[0][0:BL, 0:NCLS])
    conv1_unit(1, 1)
    score_unit(2)
    score_unit(3)

    if stage == 4:
        return finish(alphaT[0:BL, 0:NCLS])

    # ---- conv2 (relu, asp folded) -> m ----
    def conv2_unit(h, ct):
        c0, csz = C_TILES[ct]
        pg = pbig.tile([128, LH], f32, tag="big", name="pg")
        conv_taps(pg[:csz, :], [ctx_mv(dt, h) for dt in range(ND)],
                  lambda dt, k: vb("w2ctx", dt)[:, k * C + c0:k * C + c0 + csz],
                  True, False)
        conv_taps(pg[:csz, :],
                  [attT[jt][:, h * LH:(h + 1) * LH] for jt in range(2)],
                  lambda jt, k: vb("w2att", jt)[:, k * C + c0:k * C + c0 + csz],
                  False, True)
        gg = wk.tile([128, LH], bf16, tag="gg", name="gg")
        nc.scalar.activation(gg[:csz, :], pg[:csz, :], AF.Relu,
                             bias=biasv(ct, 2))
        hs = slice(h * LH, (h + 1) * LH)
        nc.vector.tensor_tensor(mT[ct][:, hs], s1T[ct][:, hs], gg[:csz, :],
                                op=ALU.mult)

    # conv2(h0) only needs attT h0 — emit before att_unit(1) so PE's
    # in-order queue isn't head-of-line blocked on softmax b2/b3.
    conv2_unit(0, 0)
    att_unit(1)
    conv2_unit(0, 1)

    if stage == 5:
        return finish(mT[0][0:BL, 0:NCLS])

    # ---- highway + maxpool ----
    def hw_unit(h, ct):
        c0, csz = C_TILES[ct]
        hs = slice(h * LH, (h + 1) * LH)
        ph = pbig.tile([128, LH], f32, tag="big", name="ph")
        for j in range(HB):
            o = h * LH + j * L1
            for jt, (j0, jsz) in enumerate(C_TILES):
                nc.tensor.matmul(ph[:csz, j * L1:(j + 1) * L1],
                                 vb("hwt", jt)[:, c0:c0 + csz],
                                 mT[jt][:, o:o + L1], start=(jt == 0),
                                 stop=(jt == len(C_TILES) - 1))
        phg = pbig.tile([128, LH], f32, tag="big", name="phg")
        for j in range(HB):
            o = h * LH + j * L1
            for jt, (j0, jsz) in enumerate(C_TILES):
                nc.tensor.matmul(phg[:csz, j * L1:(j + 1) * L1],
                                 vb("hwgt", jt)[:, c0:c0 + csz],
                                 mT[jt][:, o:o + L1], start=(jt == 0),
                                 stop=(jt == len(C_TILES) - 1))
        # per-j tail so the j0 DVE chain overlaps the j1 activations
        hh = wk.tile([128, LH], bf16, tag="hh", name="hh")
        gt = wk.tile([128, LH], bf16, tag="gt", name="gt")
        for j in range(HB):
            js = slice(j * L1, (j + 1) * L1)
            nc.scalar.activation(hh[:csz, js], ph[:csz, js], AF.Relu,
                                 bias=biasv(ct, 3))
            nc.scalar.activation(gt[:csz, js], phg[:csz, js], AF.Sigmoid,
                                 bias=biasv(ct, 4))
        for j in range(HB):
            js = slice(j * L1, (j + 1) * L1)
            ms = slice(h * LH + j * L1, h * LH + (j + 1) * L1)
            b = h * HB + j
            dd = wk.tile([128, L1], bf16, tag="dd", name="dd")
            nc.vector.tensor_tensor(dd[:csz, :], hh[:csz, js], mT[ct][:, ms],
                                    op=ALU.subtract)
            ee = wk.tile([128, L1], bf16, tag="ee", name="ee")
            nc.vector.tensor_tensor(ee[:csz, :], gt[:csz, js], dd[:csz, :],
                                    op=ALU.mult)
            m2 = wk.tile([128, L1], bf16, tag="m2", name="m2")
            nc.vector.tensor_tensor(m2[:csz, :], ee[:csz, :], mT[ct][:, ms],
                                    op=ALU.add)
            nc.vector.reduce_max(out=pooled[ct][b][:], in_=m2[:csz, :],
                                 axis=AX.X)

    po = ps2.tile([128, L2], f32, tag="sm", name="po")

    def classifier(b):
        for ct in range(len(C_TILES)):
            nc.tensor.matmul(po[0:1, b * 4:b * 4 + NCLS],
                             pooled[ct][b][:], vb("outwt", ct)[:, 0:NCLS],
                             start=(ct == 0), stop=(ct == len(C_TILES) - 1))

    hw_unit(0, 0)
    if stage == 6:
        return finish(mT[0][0:BL, 0:NCLS])
    conv2_unit(1, 0)
    hw_unit(0, 1)
    conv2_unit(1, 1)
    if stage == 7:
        return finish(mT[0][0:BL, 0:NCLS])
    for b in range(HB):
        classifier(b)
    if stage == 8:
        return finish(mT[0][0:BL, 0:NCLS])
    hw_unit(1, 0)
    hw_unit(1, 1)
    for b in range(HB, BL):
        classifier(b)
    if stage == 9:
        return finish(mT[0][0:BL, 0:NCLS])

    # out[b, c] = po[0, b*4+c] + out_b[c]
    osb = wk.tile([1, BL * NCLS], f32, tag="osb", name="osb")
    v3 = lambda ap, o: ap.rearrange("p (b x) -> p b x", x=4)[:, :, o:o + NCLS]
    nc.vector.tensor_tensor(
        osb[:].rearrange("p (b x) -> p b x", x=NCLS),
        v3(po[0:1, 0:BL * 4], 0), v3(fpk[0:1, 20:20 + BL * 4], 0), op=ALU.add)
    nc.sync.dma_start(
        d_out.ap().rearrange("(o b) c -> o (b c)", o=1), osb[:])
    stack.close()


def prep_inputs(context_ids, aspect_ids, wordemb, conv3_w, conv3_b, conv1_w,
                conv1_b, conv2_w, conv2_b, attn_W, attn_V, asp_w, asp_b, hw_w,
                hw_b, hwg_w, hwg_b, out_w, out_b):
    """Host-side prep: weight layout transforms + bf16 casts (weights only)."""
    f = np.float32
    attn_W = np.asarray(attn_W, f)
    w2 = np.asarray(conv2_w, f)
    asp_w = np.asarray(asp_w, f)

    mats = {
        "ident": np.eye(128, dtype=f),
        "w3": np.asarray(conv3_w, f).transpose(1, 2, 0).reshape(D, K * C),
        "w2t": np.ascontiguousarray(attn_W[:, D:].T),
        "w1t": np.ascontiguousarray(attn_W[:, :D].T),
        "w1c": np.asarray(conv1_w, f).transpose(1, 2, 0).reshape(D, K * C),
        "w2ctx": w2[:, :D, :].transpose(1, 2, 0).reshape(D, K * C),
        "w2att": np.einsum("aok,oc->ack", w2[:, D:, :], asp_w)
                .transpose(1, 2, 0).reshape(C, K * C),
        "hwt": np.ascontiguousarray(np.asarray(hw_w, f).T),
        "hwgt": np.ascontiguousarray(np.asarray(hwg_w, f).T),
        "outwt": np.ascontiguousarray(np.asarray(out_w, f).T),
    }
    def build_pack(offs, width):
        pk = np.zeros((128, width), np_bf16)
        for name, lst in offs.items():
            m = mats[name]
            r0 = 0
            for rows, c0, w in lst:
                pk[0:rows, c0:c0 + w] = m[r0:r0 + rows].astype(np_bf16)
                r0 += rows
        return pk

    fpk = np.zeros((128, WF), f)
    V = np.asarray(attn_V, f)[0]
    for et, (e0, esz) in enumerate(E_TILES):
        fpk[0:esz, 2 * et] = V[e0:e0 + esz]
        fpk[0:esz, 2 * et + 1] = -V[e0:e0 + esz]
    biases = np.stack([
        np.asarray(conv3_b, f),
        np.asarray(conv1_b, f),
        np.asarray(conv2_b, f) + np.einsum("aok,o->a", w2[:, D:, :],
                                           np.asarray(asp_b, f)),
        np.asarray(hw_b, f),
        np.asarray(hwg_b, f)], axis=1)
    for ct, (c0, csz) in enumerate(C_TILES):
        fpk[0:csz, 10 + 5 * ct:15 + 5 * ct] = biases[c0:c0 + csz]
    for b in range(BL):
        fpk[0, 20 + 4 * b:20 + 4 * b + NCLS] = np.asarray(out_b, f)

    shared = {
        "wordemb": np.asarray(wordemb, f).astype(np_bf16),
        "wpa": build_pack(A_OFF, WA),
        "wpb": build_pack(B_OFF, WB),
        "fpk": fpk,
    }
    in_maps = []
    for c in range(NCORES):
        m = dict(shared)
        m["ctx_ids"] = np.ascontiguousarray(
            np.asarray(context_ids, np.int32)[c * BL:(c + 1) * BL]
        ).reshape(NL, 1)
        m["asp_ids"] = np.ascontiguousarray(
            np.asarray(aspect_ids, np.int32)[c * BL:(c + 1) * BL]
        ).reshape(NM, 1)
        in_maps.append(m)
    return in_maps


def kernel(**inputs):
    if "nc" not in _NC_CACHE:
        _NC_CACHE["nc"] = build_nc()
    nc = _NC_CACHE["nc"]
    in_maps = prep_inputs(**inputs)
    res = run_bass_kernel_spmd(nc, in_maps, core_ids=list(range(NCORES)))
    return np.concatenate([res.results[c]["out"] for c in range(NCORES)], axis=0)


if __name__ == "__main__":
    print("building...")
    nc = build_nc()
    print("built ok")
